# revision 40
# baseline (speedup 1.0000x reference)
"""Trainium2 Bass kernel for the gnn_message_passing reward environment.

reference:
    diff   = feature - next_feature                    # [N, D]
    neigh  = next_action @ diff                        # [N, D]
    impact = (neigh @ neigh.T) / D                     # [N, N]
    normed = row_l2_normalize(next_feature)            # [N, D]
    sim    = normed @ normed.T                         # [N, N]
    out    = persona_a * next_action * sim             # reward_sim
           - persona_b * edges                         # reward_cost
           + persona_g * impact                        # reward_impact
    (persona_x = persona_t @ x, per-row scalars)

Distribution: 1D row shard across 8 NeuronCores (512 rows each).
All three GEMMs run fp8e4m3 DoubleRow with fp32 PSUM accumulation:
  - diff is computed on the own row shard (from 64x-prescaled bf16
    inputs), cast to fp8, and AllGathered ([N, D] fp8) -- replacing the
    16 MiB full feature/next_feature stream of the bf16 version.
  - neigh.T = diff.T @ A.T accumulates over the gathered diff; the
    result (x64) is scaled to x8 fp8 and AllGathered.
  - sim uses 16x-scaled fp8 normed (AllGathered transposed).
Masks (next_action / edges / A.T) travel as exact 0/1 fp8; the output
is written bf16 and upcast on host. Scales fold into the persona
coefficients: pa=alpha/256, pb=-beta, pg=gamma/(64*D).
"""
import numpy as np
import ml_dtypes
from contextlib import ExitStack

import concourse.bass as bass
import concourse.tile as tile
from concourse import bacc, mybir
from concourse.bass_utils import run_bass_kernel_spmd

N = 4096          # graph nodes
D = 1024          # feature dim
NPERS = 8         # personas
NCORES = 8
R = N // NCORES   # 512 rows per core
RT = R // 128     # 4 row tiles per shard
DT = D // 128     # 8 d-tiles
KT = N // 128     # 32 contraction tiles for A @ diff
NB = N // 512     # 8 output column blocks

F32 = mybir.dt.float32
BF16 = mybir.dt.bfloat16
F8 = mybir.dt.float8e4
MUL = mybir.AluOpType.mult
ADD = mybir.AluOpType.add
SUB = mybir.AluOpType.subtract
DR = mybir.MatmulPerfMode.DoubleRow


def build(reps: int = 1, stage: int = 4, mock_cc: bool = False):
    nc = bacc.Bacc("TRN2", target_bir_lowering=False, debug=False,
                   num_devices=NCORES)

    f64 = nc.dram_tensor("f64", [R, D], BF16, kind="ExternalInput").ap()
    n64 = nc.dram_tensor("n64", [R, D], BF16, kind="ExternalInput").ap()
    at = nc.dram_tensor("at", [N, R], F8, kind="ExternalInput").ap()
    amask = nc.dram_tensor("amask", [R, N], F8, kind="ExternalInput").ap()
    edges = nc.dram_tensor("edges", [R, N], F8, kind="ExternalInput").ap()
    pt = nc.dram_tensor("pt", [NPERS, R], F32, kind="ExternalInput").ap()
    gmat = nc.dram_tensor("gmat", [NPERS, 3], F32, kind="ExternalInput").ap()
    ident = nc.dram_tensor("ident", [128, 128], BF16, kind="ExternalInput").ap()
    out = nc.dram_tensor("out", [R, N], BF16, kind="ExternalOutput").ap()

    rgroups = [list(range(NCORES))]

    def blk(ap):
        """[T*128, M] -> [128, T, M] partition-tiled view."""
        return ap.rearrange("(a p) m -> p a m", p=128)

    with tile.TileContext(nc) as tc, ExitStack() as ctx:
        const = ctx.enter_context(tc.tile_pool(name="const", bufs=1))
        shard = ctx.enter_context(tc.tile_pool(name="shard", bufs=2))
        own = ctx.enter_context(tc.tile_pool(name="own", bufs=1))
        big = ctx.enter_context(tc.tile_pool(name="big", bufs=1))
        outp_pool = ctx.enter_context(tc.tile_pool(name="outp", bufs=1))
        ps = ctx.enter_context(tc.tile_pool(name="ps", bufs=8, space="PSUM"))
        dram = ctx.enter_context(tc.tile_pool(name="dram", bufs=1, space="DRAM"))

        ident_sb = const.tile([128, 128], BF16)
        nc.sync.dma_start(ident_sb[:], ident[:])
        pt_sb = const.tile([NPERS, R], F32)
        nc.sync.dma_start(pt_sb[:], pt[:])
        gmat_sb = const.tile([NPERS, 3], F32)
        nc.sync.dma_start(gmat_sb[:], gmat[:])

        for rep in range(reps):
            # ---------- collective buffers ----------
            ag_d_in = dram.tile([R, D], F8, name=f"ag_d_in{rep}", tag="agdi")
            ag_d_out = dram.tile([NCORES, R, D], F8, addr_space="Shared",
                                 name=f"ag_d_out{rep}", tag="agdo")
            ag_n_in = dram.tile([D, R], F8, name=f"ag_n_in{rep}", tag="agni")
            ag_n_out = dram.tile([NCORES, D, R], F8, addr_space="Shared",
                                 name=f"ag_n_out{rep}", tag="agno")
            ag_e_in = dram.tile([D, R], F8, name=f"ag_e_in{rep}", tag="agei")
            ag_e_out = dram.tile([NCORES, D, R], F8, addr_space="Shared",
                                 name=f"ag_e_out{rep}", tag="ageo")

            # ---------- phase 0a: diff shard -> AllGather (critical) ----------
            n_own = shard.tile([128, RT, D], BF16, name=f"n_own{rep}",
                               tag="n_own", bufs=1)
            nc.sync.dma_start(n_own[:], blk(n64))
            diff_own = own.tile([128, RT, D], F8, name=f"diff_own{rep}",
                                tag="diff_own")
            for i in range(RT):
                f_i = shard.tile([128, D], BF16, name=f"f_i{rep}_{i}",
                                 tag="f_i", bufs=2)
                nc.sync.dma_start(f_i[:], f64[i * 128:(i + 1) * 128, :])
                nc.vector.tensor_tensor(diff_own[:, i, :], f_i[:],
                                        n_own[:, i, :], SUB)
            nc.sync.dma_start(blk(ag_d_in), diff_own[:])

            if mock_cc:
                nc.sync.dma_start(ag_d_out[0][:], ag_d_in[:])
            else:
                nc.gpsimd.collective_compute(
                    "AllGather", mybir.AluOpType.bypass, ins=[ag_d_in.opt()],
                    outs=[ag_d_out.opt()], replica_groups=rgroups)

            # ---------- phase 0b: persona scalars ----------
            # pa=alpha-mix/256, pbn=-beta-mix, pgs=gamma-mix/(64*D)
            pa_sb = const.tile([128, RT], F32, name=f"pa_sb{rep}", tag="pa")
            pbn_sb = const.tile([128, RT], F32, name=f"pbn_sb{rep}", tag="pbn")
            pgs_sb = const.tile([128, RT], F32, name=f"pgs_sb{rep}", tag="pgs")
            for mt in range(RT):
                pp = ps.tile([128, 512], F32, name=f"pp{rep}_{mt}", tag="ps")
                nc.tensor.matmul(pp[:, 0:3], pt_sb[:, mt * 128:(mt + 1) * 128],
                                 gmat_sb[:], start=True, stop=True)
                nc.scalar.mul(pa_sb[:, mt:mt + 1], pp[:, 0:1], 1.0 / 256)
                nc.scalar.mul(pbn_sb[:, mt:mt + 1], pp[:, 1:2], -1.0)
                nc.scalar.mul(pgs_sb[:, mt:mt + 1], pp[:, 2:3], 1.0 / (64 * D))

            # ---------- phase 0c: normed shard -> transpose -> AllGather ------
            normedT_own = own.tile([128, DT, R], F8, name=f"ntown{rep}",
                                   tag="ntown")
            for i in range(RT):
                sq_t = shard.tile([128, D], BF16, name=f"sq_t{rep}_{i}",
                                  tag="sq_t", bufs=1)
                ss_t = shard.tile([128, 1], F32, name=f"ss_t{rep}_{i}",
                                  tag="ss_t")
                nc.scalar.activation(
                    sq_t[:], n_own[:, i, :],
                    mybir.ActivationFunctionType.Square, accum_out=ss_t[:])
                nrm_t = shard.tile([128, 1], F32, name=f"nrm_t{rep}_{i}",
                                   tag="nrm_t")
                nc.scalar.sqrt(nrm_t[:], ss_t[:])
                rn_t = shard.tile([128, 1], F32, name=f"rn_t{rep}_{i}",
                                  tag="rn_t")
                nc.vector.reciprocal(rn_t[:], nrm_t[:])
                nrmd_t = shard.tile([128, D], BF16, name=f"nrmd_t{rep}_{i}",
                                    tag="nrmd_t")
                nc.vector.tensor_scalar(nrmd_t[:], n_own[:, i, :], rn_t[:],
                                        16.0, MUL, MUL)
                for d8 in range(DT):
                    tps = ps.tile([128, 512], BF16, name=f"tps{rep}_{i}_{d8}",
                                  tag="ps")
                    nc.tensor.transpose(
                        tps[:, 0:128], nrmd_t[:, d8 * 128:(d8 + 1) * 128],
                        ident_sb[:])
                    rsl = slice(i * 128, (i + 1) * 128)
                    nc.scalar.copy(normedT_own[:, d8, rsl], tps[:, 0:128])
            nc.sync.dma_start(blk(ag_n_in), normedT_own[:])

            if mock_cc:
                nc.sync.dma_start(ag_n_out[0][:], ag_n_in[:])
            else:
                nc.gpsimd.collective_compute(
                    "AllGather", mybir.AluOpType.bypass, ins=[ag_n_in.opt()],
                    outs=[ag_n_out.opt()], replica_groups=rgroups)

            # ---------- prefetch bulk inputs ----------
            at_sb = big.tile([128, KT, R], F8, name=f"at_sb{rep}", tag="at_sb")
            nc.sync.dma_start(at_sb[:], blk(at))

            if stage <= 1:
                for d8 in range(DT):
                    nc.gpsimd.dma_start(out[0:128, d8 * 512:(d8 + 1) * 512],
                                        normedT_own[:, d8, :])
                continue

            # ---------- phase 1: neighT = diff.T @ A.T (fp8 DR, k-outer) -----
            # gathered diff arrives in 8 per-source-block DMAs that pipeline
            # under the k-sweep; all 8 psum banks accumulate concurrently.
            diff_g = big.tile([128, KT, D], F8, name=f"diff_g{rep}",
                              tag="diffg")
            for b in range(NCORES):
                nc.sync.dma_start(diff_g[:, RT * b:RT * (b + 1), :],
                                  blk(ag_d_out[b]))
            g1ps = []
            for d8 in range(DT):
                t = ps.tile([128, 512], F32, name=f"g1ps{rep}_{d8}", tag="ps")
                g1ps.append(t)
            for k2 in range(KT // 2):
                ksl = slice(2 * k2, 2 * k2 + 2)
                for d8 in range(DT):
                    nc.tensor.matmul(
                        g1ps[d8][:],
                        diff_g[:, ksl, d8 * 128:(d8 + 1) * 128],
                        at_sb[:, ksl, :],
                        start=(k2 == 0), stop=(k2 == KT // 2 - 1),
                        perf_mode=DR)
            neighT_own = own.tile([128, DT, R], F8, name=f"neown{rep}",
                                  tag="neown")
            for d8 in range(DT):
                # psum holds 64*neigh; store 8*neigh in fp8
                nc.scalar.mul(neighT_own[:, d8, :], g1ps[d8][:], 0.125)
            nc.sync.dma_start(blk(ag_e_in), neighT_own[:])

            if mock_cc:
                nc.sync.dma_start(ag_e_out[0][:], ag_e_in[:])
            else:
                nc.gpsimd.collective_compute(
                    "AllGather", mybir.AluOpType.bypass, ins=[ag_e_in.opt()],
                    outs=[ag_e_out.opt()], replica_groups=rgroups)

            if stage <= 2:
                for d8 in range(DT):
                    nc.gpsimd.dma_start(out[0:128, d8 * 512:(d8 + 1) * 512],
                                        neighT_own[:, d8, :])
                continue

            # ---------- phase 2: sim GEMM + mask*alpha ----------
            # layout [128, b*DT+d8, 512]: per-source-block d-major panels
            normedT_g = big.tile([128, NCORES * DT, R], F8, name=f"ntg{rep}",
                                 tag="ntg")
            for b in range(NCORES):
                nc.sync.dma_start(normedT_g[:, DT * b:DT * (b + 1), :],
                                  blk(ag_n_out[b]))
            outp = outp_pool.tile([128, RT, N], BF16, name=f"outp{rep}",
                                  tag="outp")
            for mt in range(RT):
                msl = slice(mt * 128, (mt + 1) * 128)
                for half in range(2):
                    am_h = shard.tile([128, N // 2],
                                      F8, name=f"am_h{rep}_{mt}_{half}",
                                      tag="am_h", bufs=2)
                    nc.sync.dma_start(
                        am_h[:], amask[msl, half * 2048:(half + 1) * 2048])
                    sps = []
                    for j in range(4):
                        t = ps.tile([128, 512], F32,
                                    name=f"sps{rep}_{mt}_{half}_{j}", tag="ps")
                        sps.append(t)
                    for k2 in range(DT // 2):
                        ksl = slice(2 * k2, 2 * k2 + 2)
                        for j in range(4):
                            nb = half * 4 + j
                            gsl = slice(nb * DT + 2 * k2, nb * DT + 2 * k2 + 2)
                            nc.tensor.matmul(
                                sps[j][:], normedT_own[:, ksl, msl],
                                normedT_g[:, gsl, :],
                                start=(k2 == 0), stop=(k2 == DT // 2 - 1),
                                perf_mode=DR)
                    for j in range(4):
                        nb = half * 4 + j
                        csl = slice(nb * 512, (nb + 1) * 512)
                        nc.vector.scalar_tensor_tensor(
                            outp[:, mt, csl], sps[j][:], pa_sb[:, mt:mt + 1],
                            am_h[:, j * 512:(j + 1) * 512], op0=MUL, op1=MUL)

            if stage <= 3:
                for mt in range(RT):
                    nc.sync.dma_start(out[mt * 128:(mt + 1) * 128, :],
                                      outp[:, mt, :])
                continue

            # ---------- phase 3: impact GEMM + combine ----------
            neighT_g = big.tile([128, NCORES * DT, R], F8, name=f"neg{rep}",
                                tag="neg")
            for b in range(NCORES):
                nc.sync.dma_start(neighT_g[:, DT * b:DT * (b + 1), :],
                                  blk(ag_e_out[b]))
            for mt in range(RT):
                msl = slice(mt * 128, (mt + 1) * 128)
                o_mt = shard.tile([128, N], BF16, name=f"o_mt{rep}_{mt}",
                                  tag="o_mt", bufs=2)
                for half in range(2):
                    ed_h = shard.tile([128, N // 2],
                                      F8, name=f"ed_h{rep}_{mt}_{half}",
                                      tag="ed_h", bufs=2)
                    nc.sync.dma_start(
                        ed_h[:], edges[msl, half * 2048:(half + 1) * 2048])
                    ips = []
                    for j in range(4):
                        t = ps.tile([128, 512], F32,
                                    name=f"ips{rep}_{mt}_{half}_{j}", tag="ps")
                        ips.append(t)
                    for k2 in range(DT // 2):
                        ksl = slice(2 * k2, 2 * k2 + 2)
                        for j in range(4):
                            nb = half * 4 + j
                            gsl = slice(nb * DT + 2 * k2, nb * DT + 2 * k2 + 2)
                            nc.tensor.matmul(
                                ips[j][:], neighT_own[:, ksl, msl],
                                neighT_g[:, gsl, :],
                                start=(k2 == 0), stop=(k2 == DT // 2 - 1),
                                perf_mode=DR)
                    for j in range(4):
                        nb = half * 4 + j
                        csl = slice(nb * 512, (nb + 1) * 512)
                        u_t = shard.tile([128, 512], BF16,
                                         name=f"u{rep}_{mt}_{half}_{j}",
                                         tag="u_t", bufs=2)
                        nc.vector.scalar_tensor_tensor(
                            u_t[:], ips[j][:], pgs_sb[:, mt:mt + 1],
                            outp[:, mt, csl], op0=MUL, op1=ADD)
                        nc.vector.scalar_tensor_tensor(
                            o_mt[:, csl], ed_h[:, j * 512:(j + 1) * 512],
                            pbn_sb[:, mt:mt + 1], u_t[:], op0=MUL, op1=ADD)
                nc.sync.dma_start(out[mt * 128:(mt + 1) * 128, :], o_mt[:])

    nc.compile()
    return nc


_CACHE = {}


def _get_nc(reps=1, stage=4, mock_cc=False):
    key = (reps, stage, mock_cc)
    if key not in _CACHE:
        _CACHE[key] = build(reps, stage, mock_cc)
    return _CACHE[key]


def make_in_maps(feature, next_feature, next_action, edges, persona_t,
                 alpha, beta, gamma):
    FP8 = ml_dtypes.float8_e4m3
    at_full = np.ascontiguousarray(np.asarray(next_action).T).astype(FP8)
    f_s = (np.asarray(feature, dtype=np.float32) * 64.0).astype(
        ml_dtypes.bfloat16)
    n_s = (np.asarray(next_feature, dtype=np.float32) * 64.0).astype(
        ml_dtypes.bfloat16)
    am8 = np.asarray(next_action).astype(FP8)
    ed8 = np.asarray(edges).astype(FP8)
    gmat = np.stack([np.asarray(alpha), np.asarray(beta),
                     np.asarray(gamma)], axis=1).astype(np.float32)
    ident = np.eye(128, dtype=ml_dtypes.bfloat16)
    in_maps = []
    for c in range(NCORES):
        rs = slice(c * R, (c + 1) * R)
        in_maps.append({
            "f64": f_s[rs],
            "n64": n_s[rs],
            "at": at_full[:, rs],
            "amask": am8[rs],
            "edges": ed8[rs],
            "pt": np.ascontiguousarray(
                np.asarray(persona_t[rs]).T).astype(np.float32),
            "gmat": gmat,
            "ident": ident,
        })
    return in_maps


def kernel(feature, next_feature, next_action, edges, persona_t,
           alpha, beta, gamma):
    nc = _get_nc(1)
    in_maps = make_in_maps(feature, next_feature, next_action, edges,
                           persona_t, alpha, beta, gamma)
    res = run_bass_kernel_spmd(nc, in_maps, list(range(NCORES)))
    return np.concatenate(
        [res.results[c]["out"].astype(np.float32) for c in range(NCORES)],
        axis=0)
